# revision 67
# baseline (speedup 1.0000x reference)
# Trainium2 Bass kernel for nn_AttentionalPropagation (B=2, D=256, N=M=4096, H=4).
#
# Sharding: 8 cores; each batch (B=2) owns 4 cores; each core computes a
# 1024-column sequence shard of the output end-to-end. k,v are computed
# redundantly per core from the full `source` of its batch. Cross-core
# communication: one AllGather of InstanceNorm partial (sum, sumsq) stats
# within each 4-core batch group (+ local sum), cheaper in the cost model
# than AllReduce.
#
# Engine plan (per core, cost-model driven):
#  - PE: all projections as fp8 DoubleRow (host-side fp8 conversion of
#    x/source/Wq/Wk/Wv), fp8 DoubleRow scores, mixed fp8-DoubleRow/bf16
#    attention, bf16 msg/h1/out. Biases enter via DVE evictions (q, v),
#    hi/lo bf16 ones-row matmul passes (b1', b2), or cancel entirely
#    (bk shifts each softmax column by a constant over m -> dropped;
#    bm is folded into b1' host-side).
#  - Softmax: scores/8 - 1 (offset keeps fp8 exp in range; cancels in the
#    normalization). exp is split between ACT (fp8 output, feeds DoubleRow
#    attn passes) and DVE (Schraudolph int16 bit-trick -> bf16 probs, feeds
#    plain bf16 attn passes). Denominator via the ones-column in v^T.
#  - Evictions that need no arithmetic go over DMA queues (msg, h1, out).

import os

import numpy as np

import concourse.bass as bass  # noqa: F401
import concourse.tile as tile
import concourse.mybir as mybir
from concourse import bacc
from concourse import bass_utils

B, D, N = 2, 256, 4096
H, DH = 4, 64
NS = N // 4           # sequence shard per core
NCORES = 8
EPS = 1e-5

FP = mybir.dt.float32
BF = mybir.dt.bfloat16
F8 = mybir.dt.float8e4
I16 = mybir.dt.int16
OP = mybir.AluOpType
AF = mybir.ActivationFunctionType
DR = mybir.MatmulPerfMode.DoubleRow

JA = 20               # exp chunks per group on ACT (fp8, DoubleRow attn)
JD = 32 - JA          # exp chunks per group on DVE (Schraudolph bf16)
# Interleave ACT/DVE ownership over the 16 m-chunk PAIRS of a group so both
# engines run concurrently (ACT pairs feed DoubleRow attn; DVE pairs bf16).
_NPAIR, _NDP = 16, JD // 2
_DVE_PAIRS = sorted({int(round((i + 0.5) * _NPAIR / _NDP - 0.5)) for i in range(_NDP)})
assert len(_DVE_PAIRS) == _NDP
C0 = 1.0              # exp offset: probs = exp(s/8 - C0); cancels in softmax
LOG2E = 1.4426950408889634
SCH_A = 0.125 * 128 * LOG2E                       # i16 = s*SCH_A + SCH_B
SCH_B = 127.0 * 128 - 128 * C0 * LOG2E - 0.5      # -0.5 centers truncation

_STAGE = os.environ.get("KSTAGE", "full")  # debug bisection: qk|attn|h1|full
_ALLGATHER = os.environ.get("KAG", "1") == "1"  # stats exchange: AllGather vs AllReduce


def _emit(nc, tc, io, es):
    out = io["out"]

    wpool = es.enter_context(tc.tile_pool(name="weights", bufs=1))
    apool = es.enter_context(tc.tile_pool(name="acts", bufs=1))

    # ---------- weight / bias / input loads ----------
    # critical path first (q/k/v projections): xs8, src8, wq, wk, wv on the
    # sync queue; everything else on the gpsimd queue.
    xs8_sb = apool.tile([128, 2, NS], F8)
    nc.sync.dma_start(out=xs8_sb[:], in_=io["xs8"].rearrange("(c p) n -> p c n", p=128))
    wq_sb = wpool.tile([128, 2, D], F8)
    nc.sync.dma_start(out=wq_sb[:], in_=io["wq8"].rearrange("(c p) o -> p c o", p=128))
    wk_sb = wpool.tile([128, 2, D], F8)
    nc.sync.dma_start(out=wk_sb[:], in_=io["wk8"].rearrange("(c p) o -> p c o", p=128))
    src8_sb = apool.tile([128, 2, N], F8)
    nc.sync.dma_start(out=src8_sb[:], in_=io["src8"].rearrange("(c p) m -> p c m", p=128))
    wv_sb = wpool.tile([128, 2, D], F8)
    nc.sync.dma_start(out=wv_sb[:], in_=io["wv8"].rearrange("(c p) o -> p c o", p=128))
    bq_sb = wpool.tile([128, 2], FP)
    nc.sync.dma_start(out=bq_sb[:], in_=io["bq"][:])
    bvb_sb = wpool.tile([128, 4, 2, DH], FP)
    nc.sync.dma_start(
        out=bvb_sb[:], in_=io["bvb"].rearrange("p (h r d) -> p h r d", h=4, r=2))

    wm_sb = wpool.tile([128, 2, D], BF)
    nc.gpsimd.dma_start(out=wm_sb[:], in_=io["wmT"].rearrange("(c p) o -> p c o", p=128))
    w1x_sb = wpool.tile([128, 2, 2 * D], BF)
    nc.gpsimd.dma_start(out=w1x_sb[:], in_=io["w1xT"].rearrange("(c p) o -> p c o", p=128))
    w1m_sb = wpool.tile([128, 2, 2 * D], BF)
    nc.gpsimd.dma_start(out=w1m_sb[:], in_=io["w1mT"].rearrange("(c p) o -> p c o", p=128))
    w2_sb = wpool.tile([128, 4, D], BF)
    nc.gpsimd.dma_start(out=w2_sb[:], in_=io["w2T"].rearrange("(c p) o -> p c o", p=128))
    b1hl_sb = wpool.tile([2, 2 * D], BF)
    nc.gpsimd.dma_start(out=b1hl_sb[:], in_=io["b1hl"][:])
    b2hl_sb = wpool.tile([2, D], BF)
    nc.gpsimd.dma_start(out=b2hl_sb[:], in_=io["b2hl"][:])
    xsb_sb = apool.tile([128, 2, NS], BF)
    nc.gpsimd.dma_start(out=xsb_sb[:], in_=io["xsb"].rearrange("(c p) n -> p c n", p=128))

    ones2 = wpool.tile([2, 512], BF)
    nc.vector.memset(ones2[:], 1.0)
    negc0 = wpool.tile([128, 1], FP)
    nc.vector.memset(negc0[:], -C0)
    eps1 = wpool.tile([128, 1], FP)
    nc.vector.memset(eps1[:], EPS)

    # ---------- persistent activation tiles ----------
    qf_sb = apool.tile([128, 2, NS], F8)      # part = 64*hh + d, [kc, n]
    kf_sb = apool.tile([128, 2, N], F8)
    # DoubleRow score layout: one 32-partition tile per (kc, hh), dims [p, r, n]
    q8_t = [[apool.tile([32, 2, NS], F8, name=f"q8_{kc}{hh}") for hh in range(2)]
            for kc in range(2)]
    k8_t = [[apool.tile([32, 2, N], F8, name=f"k8_{kc}{hh}") for hh in range(2)]
            for kc in range(2)]
    # v^T per head + ones col, fp8, stride 80 for DoubleRow
    vaT_sb = apool.tile([128, H, 16, 2, 80], F8)
    exp8_sb = apool.tile([128, 2, JA // 2, 2, 512], F8)   # [., hh, p, r, n]
    prob16_sb = apool.tile([128, 2, JD, 512], I16)        # [., hh, jd, n]
    attn_sb = apool.tile([128, 2, NS], BF)
    msg_sb = apool.tile([128, 2, NS], BF)
    h1_sb = apool.tile([128, 4, NS], FP)
    h1n_sb = apool.tile([128, 4, NS], BF)
    bst_sb = apool.tile([128, 4, 2, 6], FP)
    stats_sb = apool.tile([128, 2, 8], FP)   # per nch: [sum(4t), sumsq(4t)]

    nc.vector.memset(vaT_sb[:, :, :, :, DH:DH + 1], 1.0)

    # ---------- phase 1: projections (all fp8 DoubleRow) ----------
    with tc.tile_pool(name="pj", bufs=2, space="PSUM") as pj, \
         tc.tile_pool(name="vt", bufs=2, space="PSUM") as vtp:
        # q: per kc one DoubleRow matmul (contract 256 = 128 part x 2 ic)
        # q/k kc-major with reshuffle right after each kc so scores for the
        # first group (kc=0) can start while kc=1 and v still project.
        for kc in range(2):
            q_ps = pj.tile([128, NS], FP, tag="pj")
            for nh in range(2):
                nc.tensor.matmul(
                    q_ps[:, nh * 512:(nh + 1) * 512],
                    wq_sb[:, :, kc * 128:(kc + 1) * 128],
                    xs8_sb[:, :, nh * 512:(nh + 1) * 512],
                    start=True, stop=True, perf_mode=DR)
            nc.vector.tensor_scalar(
                out=qf_sb[:, kc, :], in0=q_ps[:],
                scalar1=bq_sb[:, kc:kc + 1], scalar2=None, op0=OP.add)
            for mq in range(4):
                k_ps = pj.tile([128, NS], FP, tag="pj")
                for mh in range(2):
                    m0 = mq * NS + mh * 512
                    nc.tensor.matmul(
                        k_ps[:, mh * 512:(mh + 1) * 512],
                        wk_sb[:, :, kc * 128:(kc + 1) * 128],
                        src8_sb[:, :, m0:m0 + 512],
                        start=True, stop=True, perf_mode=DR)
                nc.scalar.copy(kf_sb[:, kc, mq * NS:(mq + 1) * NS], k_ps[:])
            for hh in range(2):
                pi = 64 * hh
                for r in range(2):
                    nc.gpsimd.dma_start(
                        out=q8_t[kc][hh][:, r, :],
                        in_=qf_sb[pi + 32 * r:pi + 32 * r + 32, kc, :])
                    nc.gpsimd.dma_start(
                        out=k8_t[kc][hh][:, r, :],
                        in_=kf_sb[pi + 32 * r:pi + 32 * r + 32, kc, :])
        # v^T: per m-chunk of 128; out[m, (h d)]
        for g in range(8):
            vt_ps = vtp.tile([128, 2, 2, 4, DH], FP, tag="vt")
            for mm in range(4):
                mc = 4 * g + mm
                nc.tensor.matmul(
                    vt_ps[:, mm // 2, mm % 2, :, :],
                    src8_sb[:, :, mc * 128:(mc + 1) * 128], wv_sb[:],
                    start=True, stop=True, perf_mode=DR)
            for pb in range(2):
                nc.vector.tensor_tensor(
                    out=vaT_sb[:, :, 2 * g + pb, :, 0:DH],
                    in0=vt_ps[:, pb, :, :, :].transpose([0, 2, 1, 3]),
                    in1=bvb_sb[:], op=OP.add)

    if _STAGE == "qk":
        o_dbg = apool.tile([128, 2, NS], FP)
        nc.vector.tensor_copy(o_dbg[:, 0, :], qf_sb[:, 0, :])
        nc.vector.tensor_copy(o_dbg[:, 1, :], qf_sb[:, 1, :])
        nc.sync.dma_start(out=out.rearrange("(c p) n -> p c n", p=128), in_=o_dbg[:])
        return

    # ---------- phase 2+3: attention + chunkwise MLP pipeline ----------
    with tc.tile_pool(name="sc", bufs=2, space="PSUM") as scp, \
         tc.tile_pool(name="at", bufs=1, space="PSUM") as atp, \
         tc.tile_pool(name="mm", bufs=2, space="PSUM") as mmp, \
         tc.tile_pool(name="nrm", bufs=2) as nrm, \
         tc.tile_pool(name="dram", bufs=1, space="DRAM") as dram, \
         tc.tile_pool(name="nstat", bufs=1) as nstat:

        cc_in = [dram.tile([128, 8], FP, name=f"cc_in{i}") for i in range(2)]
        cc_out = [dram.tile([4, 128, 8], FP, name=f"cc_out{i}") for i in range(2)]

        def emit_stats_cc(nch):
            """Per-nch (sum, sumsq) conversion + AllGather; nch=0 is issued
            mid-kernel so its collective hides under groups 2-3."""
            for t in range(4):
                mv = nstat.tile([128, 2], FP, tag="mv")
                nc.vector.bn_aggr(out=mv[:], in_=bst_sb[:, t, nch:nch + 1, :])
                nc.vector.tensor_scalar_mul(
                    stats_sb[:, nch, t:t + 1], mv[:, 0:1], 512.0)
                msq = nstat.tile([128, 1], FP, tag="msq")
                nc.vector.tensor_mul(msq[:], mv[:, 0:1], mv[:, 0:1])
                msq2 = nstat.tile([128, 1], FP, tag="msq2")
                nc.vector.tensor_add(msq2[:], mv[:, 1:2], msq[:])
                nc.vector.tensor_scalar_mul(stats_sb[:, nch, 4 + t:5 + t], msq2[:], 512.0)
            nc.sync.dma_start(out=cc_in[nch][:], in_=stats_sb[:, nch, :])
            nc.gpsimd.collective_compute(
                "AllGather", OP.bypass,
                replica_groups=[[0, 1, 2, 3], [4, 5, 6, 7]],
                ins=[cc_in[nch][:].opt()], outs=[cc_out[nch][:].opt()],
            )

        def mlp_items(nch):
            """Deferred-emission MLP work items for column chunk `nch`,
            injected into the NEXT group's score stream so PE never idles."""
            n0 = nch * 512

            def msg_item(oc):
                def emit():
                    ps = msg_ps.pop((nch, oc), None)
                    ic0 = 1 if ps is not None else 0
                    if ps is None:
                        ps = mmp.tile([128, 512], FP, tag="mm", name=f"m_ps{oc}")
                    for ic in range(ic0, 2):
                        nc.tensor.matmul(
                            ps[:], wm_sb[:, ic, oc * 128:(oc + 1) * 128],
                            attn_sb[:, ic, n0:n0 + 512],
                            start=(ic == 0), stop=(ic == 1))
                    nc.vector.tensor_copy(msg_sb[:, oc, n0:n0 + 512], ps[:])
                return emit

            def msg_head_item(oc):
                # first contraction pass (kc=0 half), runnable one group early
                def emit():
                    ps = mmp.tile([128, 512], FP, tag="mm", name=f"m_ps{oc}")
                    msg_ps[(nch, oc)] = ps
                    nc.tensor.matmul(
                        ps[:], wm_sb[:, 0, oc * 128:(oc + 1) * 128],
                        attn_sb[:, 0, n0:n0 + 512], start=True, stop=False)
                return emit

            def h1_item(t):
                def emit():
                    h_ps = mmp.tile([128, 512], FP, tag="mm")
                    for ic in range(2):
                        nc.tensor.matmul(
                            h_ps[:], w1x_sb[:, ic, t * 128:(t + 1) * 128],
                            xsb_sb[:, ic, n0:n0 + 512], start=(ic == 0), stop=False)
                    for ic in range(2):
                        nc.tensor.matmul(
                            h_ps[:], w1m_sb[:, ic, t * 128:(t + 1) * 128],
                            msg_sb[:, ic, n0:n0 + 512], start=False, stop=False)
                    nc.tensor.matmul(
                        h_ps[:], b1hl_sb[:, t * 128:(t + 1) * 128], ones2[:],
                        start=False, stop=True)
                    nc.vector.tensor_copy(h1_sb[:, t, n0:n0 + 512], h_ps[:])
                    nc.vector.bn_stats(
                        out=bst_sb[:, t, nch, :], in_=h1_sb[:, t, n0:n0 + 512])
                return emit

            return ([msg_head_item(0), msg_head_item(1)] if nch == 1 else []), \
                [msg_item(0), msg_item(1)] + [h1_item(t) for t in range(4)]

        msg_ps = {}
        # Single deferred-work FIFO: attention passes, per-group finalizers
        # (softmax normalization + attn eviction) and MLP items all drain
        # into later groups' score streams so no engine idles at boundaries.
        work = []

        def make_pass(eng, j, o, r, ats, kc, npass):
            def emit():
                for hh in range(2):
                    h = 2 * kc + hh
                    at = ats[hh]
                    i = npass[hh]
                    npass[hh] += 1
                    last = (i == (_NPAIR - _NDP) + JD - 1)
                    if eng == "A":
                        nc.tensor.matmul(
                            at[:DH + 1, :], vaT_sb[:, h, j // 2, :, 0:DH + 1],
                            exp8_sb[:, hh, o, :, :],
                            start=(i == 0), stop=last, perf_mode=DR)
                    else:
                        nc.tensor.matmul(
                            at[:DH + 1, :], vaT_sb[:, h, j // 2, j % 2, 0:DH + 1],
                            prob16_sb[:, hh, 2 * o + r, :].bitcast(BF),
                            start=(i == 0), stop=last)
            return emit

        def make_finalize(ats, kc, n0):
            def emit():
                for hh in range(2):
                    at = ats[hh]
                    rz = nrm.tile([1, 512], FP, tag="rz")
                    nc.vector.reciprocal(rz[:], at[DH:DH + 1, :])
                    rzb = nrm.tile([DH, 512], FP, tag="rzb")
                    nc.gpsimd.partition_broadcast(rzb[:], rz[:])
                    nc.vector.tensor_mul(
                        attn_sb[64 * hh:64 * hh + DH, kc, n0:n0 + 512],
                        at[0:DH, :], rzb[:])
            return emit

        for nch in range(2):
            n0 = nch * 512
            mlp_heads, mlp_tails = mlp_items(nch)
            for kc in range(2):
                at0 = atp.tile([128, 512], FP, tag="at0")
                at1 = atp.tile([128, 512], FP, tag="at1")
                ats = (at0, at1)
                npass = [0, 0]
                sched = []  # (engine, pair t, ordinal)
                na = nd = 0
                for t in range(_NPAIR):
                    if t in _DVE_PAIRS:
                        sched.append(("D", t, nd)); nd += 1
                    else:
                        sched.append(("A", t, na)); na += 1
                # chunk-level emission order: merge the two j-streams so ACT
                # stays saturated while DVE consumes concurrently (2 PSUM bufs)
                a_js = [(2 * t + r, o, r) for e, t, o in sched if e == "A" for r in range(2)]
                d_js = [(2 * t + r, o, r) for e, t, o in sched if e == "D" for r in range(2)]
                order = []
                ca = cd = 0
                for _ in range(2 * _NPAIR):
                    if cd * 2 * JA <= ca * 2 * JD and cd < len(d_js):
                        order.append(("D",) + d_js[cd]); cd += 1
                    elif ca < len(a_js):
                        order.append(("A",) + a_js[ca]); ca += 1
                    else:
                        order.append(("D",) + d_js[cd]); cd += 1

                for eng, j, o, r in order:
                    sc_ps = scp.tile([128, 2, 512], FP, tag="sc")
                    for hh in range(2):
                        nc.tensor.matmul(
                            sc_ps[:, hh, :],
                            k8_t[kc][hh][:, :, j * 128:(j + 1) * 128],
                            q8_t[kc][hh][:, :, n0:n0 + 512],
                            start=True, stop=True, perf_mode=DR)
                    if eng == "A":
                        nc.scalar.activation(
                            out=exp8_sb[:, :, o, r, :], in_=sc_ps[:],
                            func=AF.Exp, scale=0.125, bias=negc0[:])
                        if r == 1:
                            work.append(make_pass(eng, j, o, r, ats, kc, npass))
                    else:
                        nc.vector.tensor_scalar(
                            out=prob16_sb[:, :, 2 * o + r, :], in0=sc_ps[:],
                            scalar1=SCH_A, scalar2=SCH_B, op0=OP.mult, op1=OP.add)
                        work.append(make_pass(eng, j, o, r, ats, kc, npass))
                    # drain deferred work, trailing ~3 entries behind
                    # (at most one item per chunk to avoid head-of-line stalls)
                    if len(work) > 3:
                        work.pop(0)()
                work.append(make_finalize(ats, kc, n0))
                if kc == 0:
                    work.extend(mlp_heads)
            work.extend(mlp_tails)
            if nch == 0:
                work.append(lambda: emit_stats_cc(0))
        for item in work:
            item()

        if _STAGE == "attn":
            o_dbg = apool.tile([128, 2, NS], FP)
            nc.vector.tensor_copy(o_dbg[:, 0, :], attn_sb[:, 0, :])
            nc.vector.tensor_copy(o_dbg[:, 1, :], attn_sb[:, 1, :])
            nc.sync.dma_start(out=out.rearrange("(c p) n -> p c n", p=128), in_=o_dbg[:])
            return
        if _STAGE == "h1":
            o_dbg = apool.tile([128, 2, NS], FP)
            nc.vector.tensor_copy(o_dbg[:, 0, :], h1_sb[:, 0, :])
            nc.vector.tensor_copy(o_dbg[:, 1, :], h1_sb[:, 1, :])
            nc.sync.dma_start(out=out.rearrange("(c p) n -> p c n", p=128), in_=o_dbg[:])
            return

        # ---------- instance-norm stats exchange + output ----------
        if True:
            emit_stats_cc(1)
            # one DMA pulls both collectives' results; ranks+nch innermost-ish
            sred8 = nstat.tile([128, 8, 8], FP)
            nc.sync.dma_start(
                out=sred8[:, 0:4, :], in_=cc_out[0].rearrange("k p s -> p k s"))
            nc.sync.dma_start(
                out=sred8[:, 4:8, :], in_=cc_out[1].rearrange("k p s -> p k s"))
            s02 = nstat.tile([128, 4, 8], FP)
            nc.vector.tensor_add(s02[:], sred8[:, 0:4, :], sred8[:, 4:8, :])
            s13 = nstat.tile([128, 2, 8], FP)
            nc.vector.tensor_add(s13[:], s02[:, 0:2, :], s02[:, 2:4, :])
            sred = nstat.tile([128, 8], FP)
            nc.vector.tensor_add(sred[:], s13[:, 0, :], s13[:, 1, :])

            # [mu4 | e24] = sred / N in one op; then var, rstd, -mu*rstd
            mue = nstat.tile([128, 8], FP)
            nc.vector.tensor_scalar_mul(mue[:], sred[:], 1.0 / N)
            mu4 = mue[:, 0:4]
            var4 = nstat.tile([128, 4], FP)
            nc.vector.tensor_mul(var4[:], mu4, mu4)
            nc.vector.tensor_tensor(out=var4[:], in0=mue[:, 4:8], in1=var4[:], op=OP.subtract)
            std4 = nstat.tile([128, 4], FP)
            nc.scalar.activation(out=std4[:], in_=var4[:], func=AF.Sqrt, bias=eps1[:])
            rstd4 = nstat.tile([128, 4], FP)
            nc.vector.reciprocal(rstd4[:], std4[:])
            nb4 = nstat.tile([128, 4], FP)
            nc.vector.scalar_tensor_tensor(
                out=nb4[:], in0=mu4, scalar=-1.0, in1=rstd4[:],
                op0=OP.mult, op1=OP.mult)

            # h = relu(h1 * rstd - mu * rstd), interleaved with the output
            # matmul accumulating over channel chunks t (4 PSUM tiles).
            out_sb = apool.tile([128, 2, NS], FP)
            outp = out.rearrange("(c p) n -> p c n", p=128)
            o_ps = [mmp.tile([128, 512], FP, tag="mm", name=f"o_ps{i}")
                    for i in range(2)] + \
                   [atp.tile([128, 512], FP, tag=f"at{i}", name=f"o_ps{i + 2}")
                    for i in range(2)]
            for t in range(4):
                nc.scalar.activation(
                    out=h1n_sb[:, t, :], in_=h1_sb[:, t, :], func=AF.Relu,
                    bias=nb4[:, t:t + 1], scale=rstd4[:, t:t + 1])
                for oc in range(2):
                    for nch in range(2):
                        n0 = nch * 512
                        nc.tensor.matmul(
                            o_ps[2 * oc + nch][:],
                            w2_sb[:, t, oc * 128:(oc + 1) * 128],
                            h1n_sb[:, t, n0:n0 + 512],
                            start=(t == 0), stop=False)
            for oc in range(2):
                for nch in range(2):
                    n0 = nch * 512
                    ps = o_ps[2 * oc + nch]
                    nc.tensor.matmul(
                        ps[:], b2hl_sb[:, oc * 128:(oc + 1) * 128], ones2[:],
                        start=False, stop=True)
                    nc.vector.tensor_copy(out_sb[:, oc, n0:n0 + 512], ps[:])
                    nc.sync.dma_start(out=outp[:, oc, n0:n0 + 512], in_=out_sb[:, oc, n0:n0 + 512])


_BUILT = {}


def _build():
    if "nc" in _BUILT:
        return _BUILT["nc"]
    nc = bacc.Bacc("TRN2", target_bir_lowering=False, debug=False,
                   enable_asserts=True, num_devices=NCORES)
    io = {}
    io["xs8"] = nc.dram_tensor("xs8", [D, NS], F8, kind="ExternalInput").ap()
    io["xsb"] = nc.dram_tensor("xsb", [D, NS], BF, kind="ExternalInput").ap()
    io["src8"] = nc.dram_tensor("src8", [D, N], F8, kind="ExternalInput").ap()
    io["wq8"] = nc.dram_tensor("wq8", [D, D], F8, kind="ExternalInput").ap()
    io["wk8"] = nc.dram_tensor("wk8", [D, D], F8, kind="ExternalInput").ap()
    io["wv8"] = nc.dram_tensor("wv8", [D, D], F8, kind="ExternalInput").ap()
    io["wmT"] = nc.dram_tensor("wmT", [D, D], BF, kind="ExternalInput").ap()
    io["w1xT"] = nc.dram_tensor("w1xT", [D, 2 * D], BF, kind="ExternalInput").ap()
    io["w1mT"] = nc.dram_tensor("w1mT", [D, 2 * D], BF, kind="ExternalInput").ap()
    io["w2T"] = nc.dram_tensor("w2T", [2 * D, D], BF, kind="ExternalInput").ap()
    io["bq"] = nc.dram_tensor("bq", [128, 2], FP, kind="ExternalInput").ap()
    io["bvb"] = nc.dram_tensor("bvb", [128, 8 * DH], FP, kind="ExternalInput").ap()
    io["b1hl"] = nc.dram_tensor("b1hl", [2, 2 * D], BF, kind="ExternalInput").ap()
    io["b2hl"] = nc.dram_tensor("b2hl", [2, D], BF, kind="ExternalInput").ap()
    io["out"] = nc.dram_tensor("out", [D, NS], FP, kind="ExternalOutput").ap()

    import contextlib
    with tile.TileContext(nc) as tc:
        with contextlib.ExitStack() as es:
            _emit(nc, tc, io, es)
    nc.compile()
    _BUILT["nc"] = nc
    return nc


def _prep_inputs(x, source, Wq, bq, Wk, bk, Wv, bv, Wm, bm, W1, b1, W2, b2):
    import ml_dtypes
    npF8 = mybir.dt.np(F8)
    npBF = ml_dtypes.bfloat16
    perm = np.array([4 * d + h for h in range(H) for d in range(DH)])
    f32 = lambda a: np.ascontiguousarray(a, dtype=np.float32)
    bf = lambda a: np.ascontiguousarray(np.asarray(a, np.float32), dtype=npBF)
    f8c = lambda a: np.ascontiguousarray(np.asarray(a, np.float32), dtype=npF8)

    b1p = np.asarray(b1, np.float64) + np.asarray(W1, np.float64)[:, D:] @ np.asarray(bm, np.float64)
    b1hi = np.asarray(b1p, np.float32).astype(npBF)
    b1lo = (np.asarray(b1p, np.float32) - b1hi.astype(np.float32)).astype(npBF)
    b2hi = np.asarray(b2, np.float32).astype(npBF)
    b2lo = (np.asarray(b2, np.float32) - b2hi.astype(np.float32)).astype(npBF)

    bvp = np.asarray(bv, np.float32)[perm]                    # (h, d) order
    bvb = np.tile(bvp.reshape(4, 1, DH), (1, 2, 1)).reshape(1, 8 * DH)
    bvb = np.ascontiguousarray(np.broadcast_to(bvb, (128, 8 * DH)), np.float32)

    shared = {
        "wq8": f8c(Wq[perm, :].T),
        "wk8": f8c(Wk[perm, :].T),
        "wv8": f8c(Wv[perm, :].T),
        "wmT": bf(Wm[:, perm].T),
        "w1xT": bf(W1.T[0:D, :]),
        "w1mT": bf(W1.T[D:2 * D, :]),
        "w2T": bf(W2.T),
        "bq": f32(bq[perm].reshape(2, 128).T),
        "bvb": bvb,
        "b1hl": np.ascontiguousarray(np.stack([b1hi, b1lo])),
        "b2hl": np.ascontiguousarray(np.stack([b2hi, b2lo])),
    }
    in_maps = []
    for core in range(NCORES):
        b, s = core // 4, core % 4
        m = dict(shared)
        xs = x[b][:, s * NS:(s + 1) * NS]
        m["xs8"] = f8c(xs)
        m["xsb"] = bf(xs)
        m["src8"] = f8c(source[b])
        in_maps.append(m)
    return in_maps


def run(inputs, **spmd_kwargs):
    """Build (cached), run on cores 0-7, return (full_output, BassKernelResults)."""
    nc = _build()
    in_maps = _prep_inputs(**inputs)
    res = bass_utils.run_bass_kernel_spmd(
        nc, in_maps, core_ids=list(range(NCORES)), **spmd_kwargs)
    full = np.empty((B, D, N), dtype=np.float32)
    for core in range(NCORES):
        b, s = core // 4, core % 4
        full[b][:, s * NS:(s + 1) * NS] = res.results[core]["out"]
    return full, res


def kernel(**inputs):
    full, _ = run(inputs)
    return full
